# revision 24
# baseline (speedup 1.0000x reference)
"""BehaviorAwareGCNLayer on 8 Trainium2 NeuronCores.

Math (reference):
    hx  = x @ W
    out[r] = (1/deg[r]) * sum_{e: row[e]=r} sim_w[e]*sigmoid(rep[row]+rep[col])*ns[col] * hx[col]
    out += sigmoid(rep) * (x @ W_self);  leaky_relu(out, 0.01)

Device strategy (destination sharding, no collectives):
  - By linearity, W is applied AFTER aggregation: agg[r] = sum coef_e * x[col_e],
    out[r] = (agg[r]/deg[r]) @ W + sigmoid(rep_r)*(x_r @ W_self).
  - Host does LAYOUT only (grouping/padding/fancy-index copies, one-hot
    position encoding, edge-count metadata); all value math (sigmoid,
    products, sums, matmuls) happens on device.
  - Destination rows are grouped into 32-row blocks (3125 of them); blocks
    are dealt to the 8 cores snake-wise by descending edge count, so
    per-slot capacities are nearly equal across cores -> single SPMD
    program, minimal padding. 392 block slots per core; slots 4t..4t+3
    share one [128, 64] PSUM tile (block q occupies partitions 32q..32q+31
    via the PE column-group feature).
  - Edges are grouped per (core, slot) into runs padded to whole 128-edge
    chunks, so every chunk belongs to exactly one 32-row block.
  - The host pre-expands x[col_e] (bf16) AND the 32-wide destination one-hot
    (fp8, a pure position encoding of row_e & 31) into edge-slot order, in
    batch-contiguous DRAM layout; the device STREAMS both with plain
    contiguous DMA on separate engine queues (no gather ucode at all).
  - coef = sw*sigmoid(rep_row+rep_col)*ns_col for all slots is computed once
    up front from 4 streamed bf16 metadata planes; per chunk the one-hot is
    scaled by coef (one DVE op) and used as the matmul lhsT against the raw
    streamed x rows: psum[32q+j, :] += sum_e coef_e*onehot[e,j]*x[col_e].
  - deg is shipped from host (edge bincount clamped to >=1 -- layout
    metadata); 1/deg, sigmoid(rep), and sigmoid(rep)*x_self are computed
    once up front. Finalize per quad group: one ACT copy (agg/deg -> bf16),
    two PE transposes build [agg/deg | srep*x_self]^T directly in PSUM, one
    matmul with [W; W_self] applies both weights, leaky_relu on ACT, DMA out.
    Output rows are re-assembled on host per the block permutation.
"""
import sys

if "/opt/trn_rl_repo" not in sys.path:
    sys.path.insert(0, "/opt/trn_rl_repo")

import numpy as np

P = 128
D = 64
B32 = 32                              # dest block rows
N_NODES = 100000
N_CORES = 8
N_BLK32 = N_NODES // B32              # 3125 global 32-row blocks (exact)
N_SLOT = 392                          # block slots per core (392*8=3136)
N_BLK_T = N_SLOT * N_CORES
N_GRP = N_SLOT // 4                   # 98 psum quad groups per core
BATCH = 64                            # chunks per compute batch
USE_FP8_ONEHOT = True
USE_ACT_LRELU = True


def _build_program(cap):
    """Emit + compile the single-core SPMD program. cap: [N_SLOT] run
    capacities in edges, each a multiple of 128; sum(cap) is a multiple of
    128*BATCH."""
    import concourse.bacc as bacc
    import concourse.mybir as mybir
    import concourse.tile as tile
    from concourse.masks import make_identity

    f32 = mybir.dt.float32
    bf16 = mybir.dt.bfloat16
    oh_dt = mybir.dt.float8e4 if USE_FP8_ONEHOT else bf16

    cap = [int(v) for v in cap]
    C = sum(cap) // P
    assert C % BATCH == 0
    NB = C // BATCH
    chunk_slot = []
    run_first = []
    run_last = []
    pos = 0
    for j, cp in enumerate(cap):
        nch = cp // P
        run_first.append(pos)
        run_last.append(pos + nch - 1)
        chunk_slot.extend([j] * nch)
        pos += nch
    assert pos == C

    nc = bacc.Bacc("TRN2", target_bir_lowering=False, debug=False)

    CQ = C // 4
    xexp_d = nc.dram_tensor("xexp", [NB * P, BATCH * D], bf16,
                            kind="ExternalInput")
    oh_d = nc.dram_tensor("oh", [NB * P, BATCH * B32], oh_dt,
                          kind="ExternalInput")
    meta_d = nc.dram_tensor("meta", [4 * P, 4 * CQ], bf16,
                            kind="ExternalInput")
    deg_d = nc.dram_tensor("deg", [P, N_GRP], f32, kind="ExternalInput")
    repsh_d = nc.dram_tensor("rep_sh", [P, N_GRP], f32, kind="ExternalInput")
    xself_d = nc.dram_tensor("x_self", [P, N_GRP * D], bf16,
                             kind="ExternalInput")
    wcat_d = nc.dram_tensor("w_cat", [2 * D, D], bf16, kind="ExternalInput")
    out_d = nc.dram_tensor("out", [N_GRP * P, D], f32, kind="ExternalOutput")

    AL = mybir.AluOpType
    ACT = mybir.ActivationFunctionType

    with tile.TileContext(nc) as tc:
        with (
            tc.tile_pool(name="meta", bufs=1) as meta,
            tc.tile_pool(name="metaq", bufs=2) as qpool,
            tc.tile_pool(name="gather", bufs=4) as gpool,
            tc.tile_pool(name="onehot", bufs=4) as opool,
            tc.tile_pool(name="const", bufs=1) as cpool,
            tc.tile_pool(name="fin", bufs=4) as fpool,
            tc.tile_pool(name="psum", bufs=4, space="PSUM") as psum,
            tc.tile_pool(name="psumT", bufs=2, space="PSUM") as psumT,
        ):
            deg_s = meta.tile([P, N_GRP], f32)
            repsh_s = meta.tile([P, N_GRP], f32)
            xself_s = meta.tile([P, N_GRP, D], bf16)
            coef16 = meta.tile([P, C], bf16)
            recip_all = meta.tile([P, N_GRP], f32)
            srep_all = meta.tile([P, N_GRP], f32)
            xselfS = meta.tile([P, N_GRP, D], bf16)
            wcat_s = cpool.tile([2 * D, D], bf16)
            ident = cpool.tile([P, P], bf16)
            nc.gpsimd.dma_start(out=deg_s[:], in_=deg_d[:])
            nc.gpsimd.dma_start(out=repsh_s[:], in_=repsh_d[:])
            nc.gpsimd.dma_start(out=xself_s[:].rearrange("p j d -> p (j d)"),
                                in_=xself_d[:])
            nc.gpsimd.dma_start(out=wcat_s[:], in_=wcat_d[:])
            make_identity(nc, ident[:])

            # one-time prep, interleaved with early batches: coef is computed
            # in quarters (batch 0 only needs the first quarter); the
            # finalize inputs (1/deg, sigmoid(rep), srep*x_self) must be
            # ready before batch 0's first finalize to avoid PE-queue
            # head-of-line blocking on the transposes.
            coef = meta.tile([P, C], f32)

            def coef_quarter(k):
                s = k * CQ
                e = s + CQ
                mq = qpool.tile([P, 4, CQ], bf16, tag="mq")
                nc.scalar.dma_start(out=mq[:].rearrange("p f c -> p (f c)"),
                                    in_=meta_d[k * P:(k + 1) * P, :])
                nc.vector.tensor_tensor(out=coef[:, s:e], in0=mq[:, 1, :],
                                        in1=mq[:, 2, :], op=AL.add)
                nc.scalar.activation(coef[:, s:e], coef[:, s:e], ACT.Sigmoid)
                nc.vector.tensor_tensor(out=coef[:, s:e], in0=coef[:, s:e],
                                        in1=mq[:, 0, :], op=AL.mult)
                nc.vector.tensor_tensor(out=coef16[:, s:e], in0=coef[:, s:e],
                                        in1=mq[:, 3, :], op=AL.mult)

            def xself_half(h):
                HG = N_GRP // 2
                s = h * HG
                e = N_GRP if h else HG
                nc.vector.tensor_tensor(
                    out=xselfS[:, s:e, :], in0=xself_s[:, s:e, :],
                    in1=srep_all[:, s:e].rearrange("p (j o) -> p j o", o=1)
                        .to_broadcast([P, e - s, D]),
                    op=AL.mult)

            coef_quarter(0)
            nc.vector.reciprocal(out=recip_all[:], in_=deg_s[:])
            nc.scalar.activation(srep_all[:], repsh_s[:], ACT.Sigmoid)
            xself_half(0)

            run_ps = {}  # group -> live psum tile

            def finalize_group(t):
                ps = run_ps.pop(t)
                cat0 = fpool.tile([P, D], bf16, tag="cat0")
                nc.scalar.activation(cat0[:], ps[:], ACT.Copy,
                                     scale=recip_all[:, t:t + 1])
                catT_ps = psumT.tile([P, P], bf16, tag="catT")
                nc.tensor.transpose(out=catT_ps[0:D, :], in_=cat0[:],
                                    identity=ident[:])
                nc.tensor.transpose(out=catT_ps[D:2 * D, :],
                                    in_=xselfS[:, t, :], identity=ident[:],
                                    tile_position=(0, D))
                catT = fpool.tile([P, P], bf16, tag="catT_s")
                nc.vector.tensor_copy(out=catT[:], in_=catT_ps[:])
                out_ps = psumT.tile([P, D], f32, tag="out_ps")
                nc.tensor.matmul(out=out_ps[:], lhsT=catT[:], rhs=wcat_s[:],
                                 start=True, stop=True)
                outb = fpool.tile([P, D], f32, tag="outb")
                if USE_ACT_LRELU:
                    nc.scalar.activation(outb[:], out_ps[:], ACT.Lrelu,
                                         alpha=0.01)
                else:
                    lk = fpool.tile([P, D], f32, tag="lk")
                    nc.any.tensor_scalar_mul(out=lk[:], in0=out_ps[:],
                                             scalar1=0.01)
                    nc.any.tensor_tensor(out=outb[:], in0=out_ps[:],
                                         in1=lk[:], op=AL.max)
                nc.sync.dma_start(out=out_d[t * P:(t + 1) * P, :],
                                  in_=outb[:])

            for b in range(NB):
                c0 = b * BATCH
                if b == 1:
                    xself_half(1)
                for k in range(1, 4):
                    if c0 + 2 * BATCH > k * CQ >= c0 + BATCH:
                        coef_quarter(k)
                xg = gpool.tile([P, BATCH * D], bf16, tag="xg")
                oh8 = opool.tile([P, BATCH * B32], oh_dt, tag="oh8")
                ohw = opool.tile([P, BATCH, B32], bf16, tag="ohw")
                # batch 0 is split into micro-slices so its first chunks'
                # data lands quickly despite prefetch-DMA contention
                nsub = 8 if b == 0 else 1
                SB = BATCH // nsub
                for s in range(nsub):
                    lo, hi = s * SB, (s + 1) * SB
                    nc.sync.dma_start(
                        out=xg[:, lo * D:hi * D],
                        in_=xexp_d[b * P:(b + 1) * P, lo * D:hi * D])
                    nc.scalar.dma_start(
                        out=oh8[:, lo * B32:hi * B32],
                        in_=oh_d[b * P:(b + 1) * P, lo * B32:hi * B32])
                    nc.vector.tensor_tensor(
                        out=ohw[:, lo:hi, :],
                        in0=oh8[:, lo * B32:hi * B32]
                            .rearrange("p (b n) -> p b n", n=B32),
                        in1=coef16[:, c0 + lo:c0 + hi]
                            .rearrange("p (b o) -> p b o", o=1)
                            .to_broadcast([P, hi - lo, B32]),
                        op=AL.mult)
                xg_v = xg[:].rearrange("p (b d) -> p b d", d=D)

                for i in range(BATCH):
                    ci = c0 + i
                    j = chunk_slot[ci]
                    t, q = j >> 2, j & 3
                    is_start = ci == run_first[j]
                    is_stop = ci == run_last[j]
                    if is_start and q == 0:
                        run_ps[t] = psum.tile([P, D], f32, tag="agg",
                                              name="agg_ps")
                    nc.tensor.matmul(
                        out=run_ps[t][q * B32:(q + 1) * B32, :],
                        lhsT=ohw[:, i, :], rhs=xg_v[:, i, :],
                        start=is_start, stop=is_stop,
                        tile_position=(0, q * B32))
                    if is_stop and q == 3:
                        finalize_group(t)

    nc.compile()
    return nc


def _preprocess(x, edge_index, sim_weight, rep, node_signal):
    """Host-side layout: deal destination 32-row blocks to cores (snake by
    count), group edges into (core, slot) runs padded to 128-edge chunks,
    pre-expand x[col] (bf16) and the destination one-hot (fp8) into slot
    order, produce per-core arrays + deg/rep/x_self in psum-partition
    layout."""
    import ml_dtypes

    bf16 = ml_dtypes.bfloat16
    oh_np = ml_dtypes.float8_e4m3fn if USE_FP8_ONEHOT else bf16
    row = np.ascontiguousarray(edge_index[0]).astype(np.int64)
    col = np.ascontiguousarray(edge_index[1]).astype(np.int64)
    sw = np.ascontiguousarray(sim_weight).astype(np.float32)
    rep_f = np.ascontiguousarray(rep).astype(np.float32)
    ns_f = np.ascontiguousarray(node_signal).astype(np.float32)
    x16 = np.ascontiguousarray(x).astype(bf16)
    E = row.shape[0]

    gb = row >> 5
    off = (row & 31).astype(np.int64)

    counts = np.bincount(gb, minlength=N_BLK_T).astype(np.int64)
    order_desc = np.argsort(-counts, kind="stable")
    assign = np.empty((N_CORES, N_SLOT), dtype=np.int64)
    for j in range(N_SLOT):
        ids = order_desc[j * N_CORES:(j + 1) * N_CORES]
        if j % 2 == 0:
            assign[:, j] = ids
        else:
            assign[::-1, j] = ids
    inv_core = np.empty(N_BLK_T, dtype=np.int64)
    inv_slot = np.empty(N_BLK_T, dtype=np.int64)
    for c in range(N_CORES):
        inv_core[assign[c]] = c
        inv_slot[assign[c]] = np.arange(N_SLOT)

    cap = ((counts[assign].max(axis=0) + P - 1) // P) * P
    cap = np.maximum(cap, P)
    # pad total capacity to a whole number of BATCH-chunk batches; the pad
    # goes to slot 0 so the final slots (and their finalizes) end early
    capsum = int(cap.sum())
    pad = (-capsum) % (P * BATCH)
    cap[0] += pad
    slot_base = np.zeros(N_SLOT + 1, dtype=np.int64)
    np.cumsum(cap, out=slot_base[1:])
    tot_pc = int(slot_base[-1])
    C = tot_pc // P

    core_e = inv_core[gb]
    slot_e = inv_slot[gb]
    key = core_e * N_SLOT + slot_e
    order = np.argsort(key, kind="stable")
    gcounts = np.bincount(key, minlength=N_CORES * N_SLOT)
    gstart = np.zeros(N_CORES * N_SLOT + 1, dtype=np.int64)
    np.cumsum(gcounts, out=gstart[1:])
    rank = np.arange(E, dtype=np.int64) - gstart[key[order]]
    abs_slot = core_e[order] * tot_pc + slot_base[slot_e[order]] + rank

    tot = N_CORES * tot_pc
    xexp = np.zeros((tot, D), dtype=bf16)
    xexp[abs_slot] = x16[col[order]]
    ohx = np.zeros((tot, B32), dtype=oh_np)
    ohx[abs_slot, off[order]] = oh_np(1.0)
    sw_p = np.zeros(tot, dtype=np.float32)
    rr_p = np.zeros(tot, dtype=np.float32)
    rc_p = np.zeros(tot, dtype=np.float32)
    ns_p = np.zeros(tot, dtype=np.float32)
    sw_p[abs_slot] = sw[order]
    rr_p[abs_slot] = rep_f[row[order]]
    rc_p[abs_slot] = rep_f[col[order]]
    ns_p[abs_slot] = ns_f[col[order]]

    NB = C // BATCH
    xexp_t = np.ascontiguousarray(
        xexp.reshape(N_CORES, NB, BATCH, P, D).transpose(0, 1, 3, 2, 4)
        .reshape(N_CORES, NB * P, BATCH * D))
    oh_t = np.ascontiguousarray(
        ohx.reshape(N_CORES, NB, BATCH, P, B32).transpose(0, 1, 3, 2, 4)
        .reshape(N_CORES, NB * P, BATCH * B32))

    def per_core(a):
        return a.reshape(N_CORES, C, P).transpose(0, 2, 1)

    CQ = C // 4
    meta_t = np.ascontiguousarray(
        np.stack([per_core(sw_p), per_core(rr_p), per_core(rc_p),
                  per_core(ns_p)], axis=2)           # [8, P, 4(plane), C]
        .reshape(N_CORES, P, 4, 4, CQ)               # C -> quarter, CQ
        .transpose(0, 3, 1, 2, 4)                    # [8, quarter, P, plane, CQ]
        .reshape(N_CORES, 4 * P, 4 * CQ)).astype(bf16)

    # psum-partition layout grids: row_id(c, p, t) for partition p, group t
    pj = np.arange(P) // B32
    po = np.arange(P) % B32
    slot_grid = (np.arange(N_GRP)[None, :] * 4 + pj[:, None])    # [P, N_GRP]
    gb_grid = assign[:, slot_grid]                               # [8, P, N_GRP]
    rid = gb_grid * B32 + po[None, :, None]                      # [8, P, N_GRP]
    valid = gb_grid < N_BLK32
    rid_c = np.minimum(rid, N_NODES - 1)

    degc = np.maximum(np.bincount(row, minlength=N_NODES), 1).astype(
        np.float32)
    deg_t = np.ascontiguousarray(np.where(valid, degc[rid_c], 1.0))
    repsh_t = np.ascontiguousarray(np.where(valid, rep_f[rid_c], 0.0))
    xself_t = np.ascontiguousarray(
        np.where(valid[..., None], x16[rid_c], bf16(0))
        .reshape(N_CORES, P, N_GRP * D))

    return (cap, rid, valid, xexp_t, oh_t, meta_t, deg_t, repsh_t, xself_t)


_compiled = {}


def _get_program(cap):
    key = tuple(cap.tolist())
    if key not in _compiled:
        _compiled[key] = _build_program(cap)
    return _compiled[key]


def run(x, edge_index, sim_weight, rep, node_signal, W, W_self, trace=False):
    from concourse.bass_utils import run_bass_kernel_spmd
    import ml_dtypes

    (cap, rid, valid, xexp_t, oh_t, meta_t, deg_t, repsh_t,
     xself_t) = _preprocess(x, edge_index, sim_weight, rep, node_signal)
    w_cat = np.ascontiguousarray(
        np.concatenate([np.asarray(W, dtype=np.float32),
                        np.asarray(W_self, dtype=np.float32)],
                       axis=0)).astype(ml_dtypes.bfloat16)
    nc = _get_program(cap)
    in_maps = []
    for c in range(N_CORES):
        in_maps.append({
            "xexp": xexp_t[c],
            "oh": oh_t[c],
            "meta": meta_t[c],
            "deg": deg_t[c],
            "rep_sh": repsh_t[c],
            "x_self": xself_t[c],
            "w_cat": w_cat,
        })
    res = run_bass_kernel_spmd(nc, in_maps, core_ids=list(range(N_CORES)),
                               trace=trace)
    out = np.empty((N_NODES, D), dtype=np.float32)
    for c in range(N_CORES):
        oc = res.results[c]["out"]                 # [N_GRP*P, D]
        ocv = oc.reshape(N_GRP, P, D).transpose(1, 0, 2)  # [P, N_GRP, D]
        out[rid[c][valid[c]]] = ocv[valid[c]]
    return out, res


def kernel(x, edge_index, sim_weight, rep, node_signal, W, W_self):
    out, _ = run(x, edge_index, sim_weight, rep, node_signal, W, W_self)
    return out


# revision 25
# speedup vs baseline: 1.0666x; 1.0666x over previous
"""BehaviorAwareGCNLayer on 8 Trainium2 NeuronCores.

Math (reference):
    hx  = x @ W
    out[r] = (1/deg[r]) * sum_{e: row[e]=r} sim_w[e]*sigmoid(rep[row]+rep[col])*ns[col] * hx[col]
    out += sigmoid(rep) * (x @ W_self);  leaky_relu(out, 0.01)

Device strategy (destination sharding, no collectives):
  - By linearity, W is applied AFTER aggregation: agg[r] = sum coef_e * x[col_e],
    out[r] = (agg[r]/deg[r]) @ W + sigmoid(rep_r)*(x_r @ W_self).
  - Host does LAYOUT only (grouping/padding/fancy-index copies, one-hot
    position encoding, edge-count metadata); all value math (sigmoid,
    products, sums, matmuls) happens on device.
  - Destination rows are grouped into 32-row blocks (3125 of them); blocks
    are dealt to the 8 cores snake-wise by descending edge count, so
    per-slot capacities are nearly equal across cores -> single SPMD
    program, minimal padding. 392 block slots per core; slots 4t..4t+3
    share one [128, 64] PSUM tile (block q occupies partitions 32q..32q+31
    via the PE column-group feature).
  - Edges are grouped per (core, slot) into runs padded to whole 128-edge
    chunks, so every chunk belongs to exactly one 32-row block.
  - The host pre-expands x[col_e] (bf16) AND the 32-wide destination one-hot
    (fp8, a pure position encoding of row_e & 31) into edge-slot order, in
    batch-contiguous DRAM layout; the device STREAMS both with plain
    contiguous DMA on separate engine queues (no gather ucode at all).
  - coef = sw*sigmoid(rep_row+rep_col)*ns_col for all slots is computed once
    up front from 4 streamed bf16 metadata planes; per chunk the one-hot is
    scaled by coef (one DVE op) and used as the matmul lhsT against the raw
    streamed x rows: psum[32q+j, :] += sum_e coef_e*onehot[e,j]*x[col_e].
  - deg is shipped from host (edge bincount clamped to >=1 -- layout
    metadata); 1/deg, sigmoid(rep), and sigmoid(rep)*x_self are computed
    once up front. Finalize per quad group: one ACT copy (agg/deg -> bf16),
    two PE transposes build [agg/deg | srep*x_self]^T directly in PSUM, one
    matmul with [W; W_self] applies both weights, leaky_relu on ACT, DMA out.
    Output rows are re-assembled on host per the block permutation.
"""
import sys

if "/opt/trn_rl_repo" not in sys.path:
    sys.path.insert(0, "/opt/trn_rl_repo")

import numpy as np

P = 128
D = 64
B32 = 32                              # dest block rows
N_NODES = 100000
N_CORES = 8
N_BLK32 = N_NODES // B32              # 3125 global 32-row blocks (exact)
N_SLOT = 392                          # block slots per core (392*8=3136)
N_BLK_T = N_SLOT * N_CORES
N_GRP = N_SLOT // 4                   # 98 psum quad groups per core
BATCH = 128                           # chunks per compute batch
USE_FP8_ONEHOT = True
USE_ACT_LRELU = True


def _build_program(cap):
    """Emit + compile the single-core SPMD program. cap: [N_SLOT] run
    capacities in edges, each a multiple of 128; sum(cap) is a multiple of
    128*BATCH."""
    import concourse.bacc as bacc
    import concourse.mybir as mybir
    import concourse.tile as tile
    from concourse.masks import make_identity

    f32 = mybir.dt.float32
    bf16 = mybir.dt.bfloat16
    oh_dt = mybir.dt.float8e4 if USE_FP8_ONEHOT else bf16

    cap = [int(v) for v in cap]
    C = sum(cap) // P
    assert C % BATCH == 0
    NB = C // BATCH
    chunk_slot = []
    run_first = []
    run_last = []
    pos = 0
    for j, cp in enumerate(cap):
        nch = cp // P
        run_first.append(pos)
        run_last.append(pos + nch - 1)
        chunk_slot.extend([j] * nch)
        pos += nch
    assert pos == C

    nc = bacc.Bacc("TRN2", target_bir_lowering=False, debug=False)

    CQ = C // 4
    xexp_d = nc.dram_tensor("xexp", [NB * P, BATCH * D], bf16,
                            kind="ExternalInput")
    oh_d = nc.dram_tensor("oh", [NB * P, BATCH * B32], oh_dt,
                          kind="ExternalInput")
    meta_d = nc.dram_tensor("meta", [4 * P, 4 * CQ], bf16,
                            kind="ExternalInput")
    deg_d = nc.dram_tensor("deg", [P, N_GRP], f32, kind="ExternalInput")
    repsh_d = nc.dram_tensor("rep_sh", [P, N_GRP], f32, kind="ExternalInput")
    xself_d = nc.dram_tensor("x_self", [P, N_GRP * D], bf16,
                             kind="ExternalInput")
    wcat_d = nc.dram_tensor("w_cat", [2 * D, D], bf16, kind="ExternalInput")
    out_d = nc.dram_tensor("out", [N_GRP * P, D], f32, kind="ExternalOutput")

    AL = mybir.AluOpType
    ACT = mybir.ActivationFunctionType

    with tile.TileContext(nc) as tc:
        with (
            tc.tile_pool(name="meta", bufs=1) as meta,
            tc.tile_pool(name="metaq", bufs=2) as qpool,
            tc.tile_pool(name="gather", bufs=3) as gpool,
            tc.tile_pool(name="onehot", bufs=3) as opool,
            tc.tile_pool(name="const", bufs=1) as cpool,
            tc.tile_pool(name="fin", bufs=4) as fpool,
            tc.tile_pool(name="psum", bufs=4, space="PSUM") as psum,
            tc.tile_pool(name="psumT", bufs=2, space="PSUM") as psumT,
        ):
            deg_s = meta.tile([P, N_GRP], f32)
            repsh_s = meta.tile([P, N_GRP], f32)
            xself_s = meta.tile([P, N_GRP, D], bf16)
            coef16 = meta.tile([P, C], bf16)
            recip_all = meta.tile([P, N_GRP], f32)
            srep_all = meta.tile([P, N_GRP], f32)
            xselfS = meta.tile([P, N_GRP, D], bf16)
            wcat_s = cpool.tile([2 * D, D], bf16)
            ident = cpool.tile([P, P], bf16)
            nc.gpsimd.dma_start(out=deg_s[:], in_=deg_d[:])
            nc.gpsimd.dma_start(out=repsh_s[:], in_=repsh_d[:])
            nc.gpsimd.dma_start(out=xself_s[:].rearrange("p j d -> p (j d)"),
                                in_=xself_d[:])
            nc.gpsimd.dma_start(out=wcat_s[:], in_=wcat_d[:])
            make_identity(nc, ident[:])

            # one-time prep, interleaved with early batches: coef is computed
            # in quarters (batch 0 only needs the first quarter); the
            # finalize inputs (1/deg, sigmoid(rep), srep*x_self) must be
            # ready before batch 0's first finalize to avoid PE-queue
            # head-of-line blocking on the transposes.
            coef = meta.tile([P, C], f32)

            def coef_quarter(k):
                s = k * CQ
                e = s + CQ
                mq = qpool.tile([P, 4, CQ], bf16, tag="mq")
                nc.scalar.dma_start(out=mq[:].rearrange("p f c -> p (f c)"),
                                    in_=meta_d[k * P:(k + 1) * P, :])
                nc.vector.tensor_tensor(out=coef[:, s:e], in0=mq[:, 1, :],
                                        in1=mq[:, 2, :], op=AL.add)
                nc.scalar.activation(coef[:, s:e], coef[:, s:e], ACT.Sigmoid)
                nc.vector.tensor_tensor(out=coef[:, s:e], in0=coef[:, s:e],
                                        in1=mq[:, 0, :], op=AL.mult)
                nc.vector.tensor_tensor(out=coef16[:, s:e], in0=coef[:, s:e],
                                        in1=mq[:, 3, :], op=AL.mult)

            def xself_half(h):
                HG = N_GRP // 2
                s = h * HG
                e = N_GRP if h else HG
                nc.vector.tensor_tensor(
                    out=xselfS[:, s:e, :], in0=xself_s[:, s:e, :],
                    in1=srep_all[:, s:e].rearrange("p (j o) -> p j o", o=1)
                        .to_broadcast([P, e - s, D]),
                    op=AL.mult)

            coef_quarter(0)
            nc.vector.reciprocal(out=recip_all[:], in_=deg_s[:])
            nc.scalar.activation(srep_all[:], repsh_s[:], ACT.Sigmoid)
            xself_half(0)

            run_ps = {}  # group -> live psum tile

            def finalize_group(t):
                ps = run_ps.pop(t)
                cat0 = fpool.tile([P, D], bf16, tag="cat0")
                nc.scalar.activation(cat0[:], ps[:], ACT.Copy,
                                     scale=recip_all[:, t:t + 1])
                catT_ps = psumT.tile([P, P], bf16, tag="catT")
                nc.tensor.transpose(out=catT_ps[0:D, :], in_=cat0[:],
                                    identity=ident[:])
                nc.tensor.transpose(out=catT_ps[D:2 * D, :],
                                    in_=xselfS[:, t, :], identity=ident[:],
                                    tile_position=(0, D))
                catT = fpool.tile([P, P], bf16, tag="catT_s")
                nc.vector.tensor_copy(out=catT[:], in_=catT_ps[:])
                out_ps = psumT.tile([P, D], f32, tag="out_ps")
                nc.tensor.matmul(out=out_ps[:], lhsT=catT[:], rhs=wcat_s[:],
                                 start=True, stop=True)
                outb = fpool.tile([P, D], f32, tag="outb")
                if USE_ACT_LRELU:
                    nc.scalar.activation(outb[:], out_ps[:], ACT.Lrelu,
                                         alpha=0.01)
                else:
                    lk = fpool.tile([P, D], f32, tag="lk")
                    nc.any.tensor_scalar_mul(out=lk[:], in0=out_ps[:],
                                             scalar1=0.01)
                    nc.any.tensor_tensor(out=outb[:], in0=out_ps[:],
                                         in1=lk[:], op=AL.max)
                nc.sync.dma_start(out=out_d[t * P:(t + 1) * P, :],
                                  in_=outb[:])

            for b in range(NB):
                c0 = b * BATCH
                if b == 1:
                    xself_half(1)
                for k in range(1, 4):
                    if c0 + 2 * BATCH > k * CQ >= c0 + BATCH:
                        coef_quarter(k)
                xg = gpool.tile([P, BATCH * D], bf16, tag="xg")
                oh8 = opool.tile([P, BATCH * B32], oh_dt, tag="oh8")
                ohw = opool.tile([P, BATCH, B32], bf16, tag="ohw")
                # batch 0 is split into micro-slices so its first chunks'
                # data lands quickly despite prefetch-DMA contention
                nsub = 8 if b == 0 else 1
                SB = BATCH // nsub
                for s in range(nsub):
                    lo, hi = s * SB, (s + 1) * SB
                    nc.sync.dma_start(
                        out=xg[:, lo * D:hi * D],
                        in_=xexp_d[b * P:(b + 1) * P, lo * D:hi * D])
                    nc.scalar.dma_start(
                        out=oh8[:, lo * B32:hi * B32],
                        in_=oh_d[b * P:(b + 1) * P, lo * B32:hi * B32])
                    nc.vector.tensor_tensor(
                        out=ohw[:, lo:hi, :],
                        in0=oh8[:, lo * B32:hi * B32]
                            .rearrange("p (b n) -> p b n", n=B32),
                        in1=coef16[:, c0 + lo:c0 + hi]
                            .rearrange("p (b o) -> p b o", o=1)
                            .to_broadcast([P, hi - lo, B32]),
                        op=AL.mult)
                xg_v = xg[:].rearrange("p (b d) -> p b d", d=D)

                for i in range(BATCH):
                    ci = c0 + i
                    j = chunk_slot[ci]
                    t, q = j >> 2, j & 3
                    is_start = ci == run_first[j]
                    is_stop = ci == run_last[j]
                    if is_start and q == 0:
                        run_ps[t] = psum.tile([P, D], f32, tag="agg",
                                              name="agg_ps")
                    nc.tensor.matmul(
                        out=run_ps[t][q * B32:(q + 1) * B32, :],
                        lhsT=ohw[:, i, :], rhs=xg_v[:, i, :],
                        start=is_start, stop=is_stop,
                        tile_position=(0, q * B32))
                    if is_stop and q == 3:
                        finalize_group(t)

    nc.compile()
    return nc


def _preprocess(x, edge_index, sim_weight, rep, node_signal):
    """Host-side layout: deal destination 32-row blocks to cores (snake by
    count), group edges into (core, slot) runs padded to 128-edge chunks,
    pre-expand x[col] (bf16) and the destination one-hot (fp8) into slot
    order, produce per-core arrays + deg/rep/x_self in psum-partition
    layout."""
    import ml_dtypes

    bf16 = ml_dtypes.bfloat16
    oh_np = ml_dtypes.float8_e4m3fn if USE_FP8_ONEHOT else bf16
    row = np.ascontiguousarray(edge_index[0]).astype(np.int64)
    col = np.ascontiguousarray(edge_index[1]).astype(np.int64)
    sw = np.ascontiguousarray(sim_weight).astype(np.float32)
    rep_f = np.ascontiguousarray(rep).astype(np.float32)
    ns_f = np.ascontiguousarray(node_signal).astype(np.float32)
    x16 = np.ascontiguousarray(x).astype(bf16)
    E = row.shape[0]

    gb = row >> 5
    off = (row & 31).astype(np.int64)

    counts = np.bincount(gb, minlength=N_BLK_T).astype(np.int64)
    order_desc = np.argsort(-counts, kind="stable")
    assign = np.empty((N_CORES, N_SLOT), dtype=np.int64)
    for j in range(N_SLOT):
        ids = order_desc[j * N_CORES:(j + 1) * N_CORES]
        if j % 2 == 0:
            assign[:, j] = ids
        else:
            assign[::-1, j] = ids
    inv_core = np.empty(N_BLK_T, dtype=np.int64)
    inv_slot = np.empty(N_BLK_T, dtype=np.int64)
    for c in range(N_CORES):
        inv_core[assign[c]] = c
        inv_slot[assign[c]] = np.arange(N_SLOT)

    cap = ((counts[assign].max(axis=0) + P - 1) // P) * P
    cap = np.maximum(cap, P)
    # pad total capacity to a whole number of BATCH-chunk batches; the pad
    # goes to slot 0 so the final slots (and their finalizes) end early
    capsum = int(cap.sum())
    pad = (-capsum) % (P * BATCH)
    cap[0] += pad
    slot_base = np.zeros(N_SLOT + 1, dtype=np.int64)
    np.cumsum(cap, out=slot_base[1:])
    tot_pc = int(slot_base[-1])
    C = tot_pc // P

    core_e = inv_core[gb]
    slot_e = inv_slot[gb]
    key = core_e * N_SLOT + slot_e
    order = np.argsort(key, kind="stable")
    gcounts = np.bincount(key, minlength=N_CORES * N_SLOT)
    gstart = np.zeros(N_CORES * N_SLOT + 1, dtype=np.int64)
    np.cumsum(gcounts, out=gstart[1:])
    rank = np.arange(E, dtype=np.int64) - gstart[key[order]]
    abs_slot = core_e[order] * tot_pc + slot_base[slot_e[order]] + rank

    tot = N_CORES * tot_pc
    xexp = np.zeros((tot, D), dtype=bf16)
    xexp[abs_slot] = x16[col[order]]
    ohx = np.zeros((tot, B32), dtype=oh_np)
    ohx[abs_slot, off[order]] = oh_np(1.0)
    sw_p = np.zeros(tot, dtype=np.float32)
    rr_p = np.zeros(tot, dtype=np.float32)
    rc_p = np.zeros(tot, dtype=np.float32)
    ns_p = np.zeros(tot, dtype=np.float32)
    sw_p[abs_slot] = sw[order]
    rr_p[abs_slot] = rep_f[row[order]]
    rc_p[abs_slot] = rep_f[col[order]]
    ns_p[abs_slot] = ns_f[col[order]]

    NB = C // BATCH
    xexp_t = np.ascontiguousarray(
        xexp.reshape(N_CORES, NB, BATCH, P, D).transpose(0, 1, 3, 2, 4)
        .reshape(N_CORES, NB * P, BATCH * D))
    oh_t = np.ascontiguousarray(
        ohx.reshape(N_CORES, NB, BATCH, P, B32).transpose(0, 1, 3, 2, 4)
        .reshape(N_CORES, NB * P, BATCH * B32))

    def per_core(a):
        return a.reshape(N_CORES, C, P).transpose(0, 2, 1)

    CQ = C // 4
    meta_t = np.ascontiguousarray(
        np.stack([per_core(sw_p), per_core(rr_p), per_core(rc_p),
                  per_core(ns_p)], axis=2)           # [8, P, 4(plane), C]
        .reshape(N_CORES, P, 4, 4, CQ)               # C -> quarter, CQ
        .transpose(0, 3, 1, 2, 4)                    # [8, quarter, P, plane, CQ]
        .reshape(N_CORES, 4 * P, 4 * CQ)).astype(bf16)

    # psum-partition layout grids: row_id(c, p, t) for partition p, group t
    pj = np.arange(P) // B32
    po = np.arange(P) % B32
    slot_grid = (np.arange(N_GRP)[None, :] * 4 + pj[:, None])    # [P, N_GRP]
    gb_grid = assign[:, slot_grid]                               # [8, P, N_GRP]
    rid = gb_grid * B32 + po[None, :, None]                      # [8, P, N_GRP]
    valid = gb_grid < N_BLK32
    rid_c = np.minimum(rid, N_NODES - 1)

    degc = np.maximum(np.bincount(row, minlength=N_NODES), 1).astype(
        np.float32)
    deg_t = np.ascontiguousarray(np.where(valid, degc[rid_c], 1.0))
    repsh_t = np.ascontiguousarray(np.where(valid, rep_f[rid_c], 0.0))
    xself_t = np.ascontiguousarray(
        np.where(valid[..., None], x16[rid_c], bf16(0))
        .reshape(N_CORES, P, N_GRP * D))

    return (cap, rid, valid, xexp_t, oh_t, meta_t, deg_t, repsh_t, xself_t)


_compiled = {}


def _get_program(cap):
    key = tuple(cap.tolist())
    if key not in _compiled:
        _compiled[key] = _build_program(cap)
    return _compiled[key]


def run(x, edge_index, sim_weight, rep, node_signal, W, W_self, trace=False):
    from concourse.bass_utils import run_bass_kernel_spmd
    import ml_dtypes

    (cap, rid, valid, xexp_t, oh_t, meta_t, deg_t, repsh_t,
     xself_t) = _preprocess(x, edge_index, sim_weight, rep, node_signal)
    w_cat = np.ascontiguousarray(
        np.concatenate([np.asarray(W, dtype=np.float32),
                        np.asarray(W_self, dtype=np.float32)],
                       axis=0)).astype(ml_dtypes.bfloat16)
    nc = _get_program(cap)
    in_maps = []
    for c in range(N_CORES):
        in_maps.append({
            "xexp": xexp_t[c],
            "oh": oh_t[c],
            "meta": meta_t[c],
            "deg": deg_t[c],
            "rep_sh": repsh_t[c],
            "x_self": xself_t[c],
            "w_cat": w_cat,
        })
    res = run_bass_kernel_spmd(nc, in_maps, core_ids=list(range(N_CORES)),
                               trace=trace)
    out = np.empty((N_NODES, D), dtype=np.float32)
    for c in range(N_CORES):
        oc = res.results[c]["out"]                 # [N_GRP*P, D]
        ocv = oc.reshape(N_GRP, P, D).transpose(1, 0, 2)  # [P, N_GRP, D]
        out[rid[c][valid[c]]] = ocv[valid[c]]
    return out, res


def kernel(x, edge_index, sim_weight, rep, node_signal, W, W_self):
    out, _ = run(x, edge_index, sim_weight, rep, node_signal, W, W_self)
    return out
